# revision 33
# baseline (speedup 1.0000x reference)
"""Distributed Trainium2 (Bass/Tile) kernel for a batched quantized matmul.

Reference computation (all shapes hardcoded):
    out[s,b,m,n] = sum_k (x[s,b,m,k] + 66)*0.03 * (y[b,k,n] - 160)*0.025
    x: [7, 8, 1024, 1024] f32 holding ints in [-128, 127]
    y: [8, 1024, 1024]    f32 holding ints in [0, 255]
    out: [7, 8, 1024, 1024] f32

Sharding: data-parallel over B=8 -> one batch element b per NeuronCore.
Core b gets x[:, b] and y[b]; no collectives needed.

Device kernel (per core), fp8 DoubleRow variant:
  - The rel-err gate is 2e-2; quantizing the zero-point-shifted operands
    (x+66 in [-62,193], y-160 in [-160,95]) to TRN fp8e4 (e4m3, max 240)
    costs ~4.6e-3 rel err (validated in numpy) -- well inside the gate.
  - fp8e4 matmuls with perf_mode=DoubleRow contract 256 k-elements per
    instruction (2 multiplies/cell/cycle): half the bf16 instruction
    count for the same work. Operand tiles are [128, 2, F]: partition
    dim holds k%128, dim1 the two k-subtiles, F the free dim.
  - Host pre-applies the zero points during the fp8 cast, so the device
    does no dequant arithmetic at all; the combined scale
    0.03*0.025 = 7.5e-4 is fused into the PSUM->SBUF eviction copy.
  - Output is stored bf16 (halves out-DMA; +2e-4 rel err) and upcast to
    f32 on the host.
"""

import numpy as np
import ml_dtypes

import concourse.bass as bass
import concourse.mybir as mybir
from concourse import bacc
from concourse.tile import TileContext
from concourse.bass_utils import run_bass_kernel_spmd

S, B, M, K, N = 7, 8, 1024, 1024, 1024
P = 128          # SBUF partitions / PE array dim
NB = 512         # one PSUM bank of fp32
KP = 2 * P       # k-elements contracted per DoubleRow matmul
KTT, MTT = K // KP, M // P  # 4, 8 (host-side tiling of the x layout)
X_ZP = -66.0
Y_ZP = 160.0
OUT_SCALE = 0.03 * 0.025
BF16 = mybir.dt.bfloat16
FP8 = mybir.dt.float8e4
F32 = mybir.dt.float32
ACT_COPY = mybir.ActivationFunctionType.Copy
DOUBLE_ROW = mybir.MatmulPerfMode.DoubleRow
DR_SW = mybir.MatmulPerfMode.DoubleRowSwInterleave

_CACHED_NC = None


def build():
    # Bacc (not plain Bass): its finalize() runs generate_event_semaphores,
    # which splits multi-wait sync_info to the <=1-wait-per-instruction HW
    # limit (walrus rejects the unsplit form with "Too many sync waits").
    nc = bacc.Bacc("TRN2", target_bir_lowering=False)
    KT, MT, NT = K // KP, M // P, N // NB  # 4, 8, 2
    # x is provided per (s, ki2) in DoubleRowSwInterleave weight layout:
    # x_d[s, ki2, p, mj*256 + 2*j + i] = xq[m = mj*128 + 127 - j,
    #                                       k = ki2*256 + i*128 + p]
    # so each weight tile is one contiguous 256B/partition LDWEIGHTS read.
    x_d = nc.declare_dram_parameter("x", [S, KT, P, MT * 2 * P], FP8,
                                    isOutput=False)
    # y is provided pre-tiled per ki2: y_d[ki2, p, i, n] = yq[ki2*256+i*128+p, n]
    # so each y tile is a single contiguous [128, 2048B] DMA (dma_start
    # issue costs ~0.7us of sequencer time each; fewer, bigger DMAs get
    # the first operands on-chip sooner).
    y_d = nc.declare_dram_parameter("y", [KT, P, 2, N], FP8, isOutput=False)
    o_d = nc.declare_dram_parameter("out", [S, M, N], BF16, isOutput=True)

    with TileContext(nc) as tc:
        with tc.tile_pool(name="ypool", bufs=1) as ypool, \
             tc.tile_pool(name="xpool", bufs=2 * KT) as xpool, \
             tc.tile_pool(name="pspool", bufs=4, space="PSUM") as pspool, \
             tc.tile_pool(name="opool", bufs=6) as opool:
            # Warm-up: the PE HAM clock gate holds the array at 1.2 GHz
            # until it sees ~3.4us of sustained activity. Burn part of that
            # window on dummy matmuls while the first operand DMAs are in
            # flight. Only one column is memset (tile allocation needs a
            # producer); the rest is read as garbage, which is fine: the PE
            # has no traps, the warm PSUM bank is never read, and the first
            # real matmul's start=True resets it. This keeps the warm-up
            # off the critical path of a full-tile memset.
            warm_src = ypool.tile([P, NB], BF16, tag="warmsrc")
            nc.vector.memset(warm_src[:, 0:1], 1.0)
            warm_ps = pspool.tile([P, N], F32, tag="ps", name="warm")
            for _ in range(4):
                nc.tensor.matmul(warm_ps[:, 0:NB], warm_src[:, 0:P],
                                 warm_src[:], start=True, stop=True)

            # Load y (zero-point pre-applied on host) as 4 [128, 2, N]
            # DoubleRow tiles on the sync queue, while the s=0 x chunks
            # issue in parallel on the gpsimd queue: each queue's sequencer
            # serializes dma_start processing at ~0.7us per transfer, so
            # splitting the ki-ordered operand pairs across two queues
            # halves the time until the first matmul group's operands land.
            yq = [None] * KT
            xT0 = [None] * KT
            for ki in range(KT):
                yt = ypool.tile([P, 2, N], FP8, tag=f"y{ki}")
                nc.sync.dma_start(out=yt[:], in_=y_d[ki])
                yq[ki] = yt
                # s=0 x rides the scalar HWDGE queue (idle until the first
                # evictions ~8us later): the gpsimd SWDGE path is slow to
                # start, which would delay the first real matmuls.
                xt = xpool.tile([P, MT, 2 * P], FP8, tag="xT", name="xt0")
                nc.scalar.dma_start(out=xt[:], in_=x_d[0, ki])
                xT0[ki] = xt

            def evict(ot_sl, ps_sl, odd):
                # PSUM -> SBUF bf16 with fused scale, alternating between
                # the Scalar and Vector engines so neither eviction queue
                # accumulates backlog against the PE stream (a single queue
                # carrying all 57 x ~1.26us evictions plus issue overhead
                # runs within ~5% of the whole kernel span).
                if odd:
                    nc.vector.tensor_scalar_mul(ot_sl, ps_sl, OUT_SCALE)
                else:
                    nc.scalar.activation(ot_sl, ps_sl, ACT_COPY,
                                         scale=OUT_SCALE)

            def store(dram_sl, ot_sl, odd):
                # store issues ride the near-idle sync queue: the ~0.7us
                # dma_start sequencer cost plus the ~0.75us cross-queue
                # wait fit easily there, and the store is off the
                # PSUM-recycle critical path (it only reads the SBUF copy)
                nc.sync.dma_start(out=dram_sl, in_=ot_sl)

            def mj_group(s, mj, xT, odd, split_evict=False):
                """One output stripe [128, 1024]: ki-inner accumulation into
                a 2-bank PSUM tile, then a single eviction + store. For the
                very last group, evict/store per nj half instead so the nj=0
                half drains while nj=1's final matmuls still stream."""
                pst = pspool.tile([P, N], F32, tag="ps", name="ps")
                for ki in range(KT):
                    lhsT = xT[ki][:, mj, :]
                    for nj in range(NT):
                        nc.tensor.matmul(
                            pst[:, nj * NB:(nj + 1) * NB], lhsT,
                            yq[ki][:, :, nj * NB:(nj + 1) * NB],
                            start=(ki == 0), stop=(ki == KT - 1),
                            perf_mode=DR_SW)
                ot = opool.tile([P, N], BF16, tag="o", name="ot")
                if split_evict:
                    # last stripe: drain the two nj halves on the two
                    # evictor queues in parallel
                    for nj in range(NT):
                        sl = slice(nj * NB, (nj + 1) * NB)
                        evict(ot[:, sl], pst[:, sl], nj % 2)
                        store(o_d[s, mj * P:(mj + 1) * P, sl], ot[:, sl],
                              nj % 2)
                else:
                    evict(ot[:], pst[:], odd)
                    store(o_d[s, mj * P:(mj + 1) * P, :], ot[:], odd)

            for s in range(S):
                if s == 0:
                    xT = xT0
                    # Startup: operands arrive at DMA rate; consume each ki
                    # chunk for two mj stripes as it lands (ki-outer, 2 open
                    # groups — same interleaving degree as the plain loop).
                    MJ_HEAD = 2
                    head = [pspool.tile([P, N], F32, tag="ps", name=f"ph{mj}")
                            for mj in range(MJ_HEAD)]
                    for ki in range(KT):
                        for mj in range(MJ_HEAD):
                            lhsT = xT[ki][:, mj, :]
                            for nj in range(NT):
                                nc.tensor.matmul(
                                    head[mj][:, nj * NB:(nj + 1) * NB], lhsT,
                                    yq[ki][:, :, nj * NB:(nj + 1) * NB],
                                    start=(ki == 0), stop=(ki == KT - 1),
                                    perf_mode=DR_SW)
                    for mj in range(MJ_HEAD):
                        ot = opool.tile([P, N], BF16, tag="o", name="oth")
                        evict(ot[:], head[mj][:], mj % 2)
                        store(o_d[0, mj * P:(mj + 1) * P, :], ot[:], mj % 2)
                    for mj in range(MJ_HEAD, MT):
                        mj_group(s, mj, xT, mj % 2)
                    continue
                else:
                    if s == 1:
                        # Gate the bulk x prefetch behind the last y tile's
                        # arrival: otherwise 3MB of prefetch is in flight at
                        # t=0 and the DMA rings fair-share bandwidth, so the
                        # first-needed s=0 tiles complete ~2us later.
                        gate = ypool.tile([P, 4], FP8, tag="gate")
                        nc.gpsimd.tensor_copy(gate[:], yq[KT - 1][:, 0, 0:4])
                    xT = []
                    for ki in range(KT):
                        xt = xpool.tile([P, MT, 2 * P], FP8, tag="xT")
                        nc.gpsimd.dma_start(out=xt[:], in_=x_d[s, ki])
                        xT.append(xt)
                for mj in range(MT):
                    mj_group(s, mj, xT, mj % 2,
                             split_evict=(s == S - 1 and mj == MT - 1))
    nc.finalize()
    return nc


def _shard_inputs(x, y):
    f8 = ml_dtypes.float8_e4m3
    in_maps = []
    for b in range(B):
        # zero points pre-applied; |values| <= 193 fit e4m3 (max 240)
        # with <= 6.25% per-element rounding error -> ~4.6e-3 rel err.
        # x shard: k-major transpose, then the DoubleRowSwInterleave weight
        # layout (see build()): per (s, ki2, mj) block of 256, position
        # 2*j + i holds column (127 - j) of k-subtile i.
        xq = (np.ascontiguousarray(x[:, b].transpose(0, 2, 1))
              - np.float32(X_ZP)).astype(f8)          # [S, K, M]
        a = xq.reshape(S, KTT, 2, P, MTT, P)          # [s, ki2, i, p, mj, j]
        a = a.transpose(0, 1, 3, 4, 5, 2)[:, :, :, :, ::-1, :]
        # y: per-ki2 DoubleRow tile layout [ki2, p, i, n] (one DMA per tile)
        yq = (y[b] - np.float32(Y_ZP)).astype(f8)    # [K, N]
        yq = yq.reshape(KTT, 2, P, N).transpose(0, 2, 1, 3)
        in_maps.append({
            "x": np.ascontiguousarray(a).reshape(S, KTT, P, MTT * 2 * P),
            "y": np.ascontiguousarray(yq),
        })
    return in_maps


def run(x, y, trace=False):
    global _CACHED_NC
    if _CACHED_NC is None:
        _CACHED_NC = build()
    nc = _CACHED_NC
    in_maps = _shard_inputs(x, y)
    res = run_bass_kernel_spmd(nc, in_maps, core_ids=list(range(B)), trace=trace)
    out = np.stack([np.asarray(res.results[b]["out"]) for b in range(B)], axis=1)
    return out.astype(np.float32), res


def kernel(x, y):
    out, _ = run(x, y, trace=False)
    return out


# revision 37
# speedup vs baseline: 1.0116x; 1.0116x over previous
"""Distributed Trainium2 (Bass/Tile) kernel for a batched quantized matmul.

Reference computation (all shapes hardcoded):
    out[s,b,m,n] = sum_k (x[s,b,m,k] + 66)*0.03 * (y[b,k,n] - 160)*0.025
    x: [7, 8, 1024, 1024] f32 holding ints in [-128, 127]
    y: [8, 1024, 1024]    f32 holding ints in [0, 255]
    out: [7, 8, 1024, 1024] f32

Sharding: data-parallel over B=8 -> one batch element b per NeuronCore.
Core b gets x[:, b] and y[b]; no collectives needed.

Device kernel (per core), fp8 DoubleRow variant (116us vs 216us bf16):
  - The rel-err gate is 2e-2; quantizing the zero-point-shifted operands
    (x+66 in [-62,193], y-160 in [-160,95]) to TRN fp8e4 (e4m3, max 240)
    costs 4.8e-3 rel err (validated in numpy AND on hw) -- well inside
    the gate. Host pre-applies the zero points during the fp8 cast, so
    the device does no dequant arithmetic at all; the combined scale
    0.03*0.025 = 7.5e-4 is fused into the PSUM->SBUF eviction.
  - fp8e4 matmuls in DoubleRow mode contract 256 k-elements per
    instruction (2 multiplies/cell/cycle): half the bf16 instruction
    count for the same work. 448 MMs x 213ns = 95.5us PE floor; the
    kernel streams them back-to-back at that rate (measured).
  - Plain DoubleRow ran MMs at 259ns: the 256-column non-contiguous
    LDWEIGHTS stole the rhs stream's SBUF/XBUS bandwidth. With
    DoubleRowSwInterleave the host pre-interleaves each weight tile
    into one contiguous 256B/partition block; LDWEIGHTS (130ns) then
    overlaps 100% and MMs hit the 213ns roofline.
  - Eviction alternates ScalarE/DVE per stripe and store issues ride
    the sync queue: one queue cannot hold 57 x 1.26us evictions plus
    57 x 0.7us dma_start issue slots inside the PE span.
  - Output is stored bf16 (halves out-DMA; +2e-4 rel err) and upcast
    to f32 on the host.
  - Remaining overhead vs the 95.5us floor: ~4.5us semaphore-init
    prologue, ~4us DMA-start/fair-share latency before the first real
    MM (cold-clock matmuls fill it), ~6us eviction drain + teardown.
"""

import numpy as np
import ml_dtypes

import concourse.bass as bass
import concourse.mybir as mybir
from concourse import bacc
from concourse.tile import TileContext
from concourse.bass_utils import run_bass_kernel_spmd

S, B, M, K, N = 7, 8, 1024, 1024, 1024
P = 128          # SBUF partitions / PE array dim
NB = 512         # one PSUM bank of fp32
KP = 2 * P       # k-elements contracted per DoubleRow matmul
KTT, MTT = K // KP, M // P  # 4, 8 (host-side tiling of the x layout)
X_ZP = -66.0
Y_ZP = 160.0
OUT_SCALE = 0.03 * 0.025
BF16 = mybir.dt.bfloat16
FP8 = mybir.dt.float8e4
F32 = mybir.dt.float32
ACT_COPY = mybir.ActivationFunctionType.Copy
DR_SW = mybir.MatmulPerfMode.DoubleRowSwInterleave

_CACHED_NC = None


def build():
    # Bacc (not plain Bass): its finalize() runs generate_event_semaphores,
    # which splits multi-wait sync_info to the <=1-wait-per-instruction HW
    # limit (walrus rejects the unsplit form with "Too many sync waits").
    nc = bacc.Bacc("TRN2", target_bir_lowering=False)
    KT, MT, NT = K // KP, M // P, N // NB  # 4, 8, 2
    # x is provided per (s, ki2) in DoubleRowSwInterleave weight layout:
    # x_d[s, ki2, p, mj*256 + 2*j + i] = xq[m = mj*128 + 127 - j,
    #                                       k = ki2*256 + i*128 + p]
    # so each weight tile is one contiguous 256B/partition LDWEIGHTS read.
    x_d = nc.declare_dram_parameter("x", [S, KT, P, MT * 2 * P], FP8,
                                    isOutput=False)
    # y is provided pre-tiled per ki2: y_d[ki2, p, i, n] = yq[ki2*256+i*128+p, n]
    # so each y tile is a single contiguous [128, 2048B] DMA (dma_start
    # issue costs ~0.7us of sequencer time each; fewer, bigger DMAs get
    # the first operands on-chip sooner).
    y_d = nc.declare_dram_parameter("y", [KT, P, 2, N], FP8, isOutput=False)
    o_d = nc.declare_dram_parameter("out", [S, M, N], BF16, isOutput=True)

    with TileContext(nc) as tc:
        with tc.tile_pool(name="ypool", bufs=1) as ypool, \
             tc.tile_pool(name="xpool", bufs=2 * KT) as xpool, \
             tc.tile_pool(name="pspool", bufs=4, space="PSUM") as pspool, \
             tc.tile_pool(name="opool", bufs=6) as opool:
            # Warm-up: the PE HAM clock gate holds the array at 1.2 GHz
            # until it sees ~3.4us of sustained activity. Burn part of that
            # window on dummy matmuls while the first operand DMAs are in
            # flight. Only one column is memset (tile allocation needs a
            # producer); the rest is read as garbage, which is fine: the PE
            # has no traps, the warm PSUM bank is never read, and the first
            # real matmul's start=True resets it. This keeps the warm-up
            # off the critical path of a full-tile memset.
            warm_src = ypool.tile([P, NB], BF16, tag="warmsrc")
            nc.vector.memset(warm_src[:, 0:1], 1.0)
            warm_ps = pspool.tile([P, N], F32, tag="ps", name="warm")
            for _ in range(4):
                nc.tensor.matmul(warm_ps[:, 0:NB], warm_src[:, 0:P],
                                 warm_src[:], start=True, stop=True)

            # Load y (zero-point pre-applied on host) as 4 [128, 2, N]
            # DoubleRow tiles on the sync queue, while the s=0 x chunks
            # issue in parallel on the gpsimd queue: each queue's sequencer
            # serializes dma_start processing at ~0.7us per transfer, so
            # splitting the ki-ordered operand pairs across two queues
            # halves the time until the first matmul group's operands land.
            yq = [None] * KT
            xT0 = [None] * KT
            for ki in range(KT):
                yt = ypool.tile([P, 2, N], FP8, tag=f"y{ki}")
                nc.sync.dma_start(out=yt[:], in_=y_d[ki])
                yq[ki] = yt
                xt = xpool.tile([P, MT, 2 * P], FP8, tag="xT", name="xt0")
                nc.gpsimd.dma_start(out=xt[:], in_=x_d[0, ki])
                xT0[ki] = xt

            def evict(ot_sl, ps_sl, odd):
                # PSUM -> SBUF bf16 with fused scale, alternating between
                # the Scalar and Vector engines so neither eviction queue
                # accumulates backlog against the PE stream (a single queue
                # carrying all 57 x ~1.26us evictions plus issue overhead
                # runs within ~5% of the whole kernel span).
                if odd:
                    nc.vector.tensor_scalar_mul(ot_sl, ps_sl, OUT_SCALE)
                else:
                    nc.scalar.activation(ot_sl, ps_sl, ACT_COPY,
                                         scale=OUT_SCALE)

            def store(dram_sl, ot_sl, odd):
                # store issues ride the near-idle sync queue: the ~0.7us
                # dma_start sequencer cost plus the ~0.75us cross-queue
                # wait fit easily there, and the store is off the
                # PSUM-recycle critical path (it only reads the SBUF copy)
                nc.sync.dma_start(out=dram_sl, in_=ot_sl)

            def mj_group(s, mj, xT, odd, split_evict=False):
                """One output stripe [128, 1024]: ki-inner accumulation into
                a 2-bank PSUM tile, then a single eviction + store. For the
                very last group, evict/store per nj half instead so the nj=0
                half drains while nj=1's final matmuls still stream."""
                pst = pspool.tile([P, N], F32, tag="ps", name="ps")
                for ki in range(KT):
                    lhsT = xT[ki][:, mj, :]
                    for nj in range(NT):
                        nc.tensor.matmul(
                            pst[:, nj * NB:(nj + 1) * NB], lhsT,
                            yq[ki][:, :, nj * NB:(nj + 1) * NB],
                            start=(ki == 0), stop=(ki == KT - 1),
                            perf_mode=DR_SW)
                ot = opool.tile([P, N], BF16, tag="o", name="ot")
                if split_evict:
                    # last stripe: drain the two nj halves on the two
                    # evictor queues in parallel
                    for nj in range(NT):
                        sl = slice(nj * NB, (nj + 1) * NB)
                        evict(ot[:, sl], pst[:, sl], nj % 2)
                        store(o_d[s, mj * P:(mj + 1) * P, sl], ot[:, sl],
                              nj % 2)
                else:
                    evict(ot[:], pst[:], odd)
                    store(o_d[s, mj * P:(mj + 1) * P, :], ot[:], odd)

            for s in range(S):
                if s == 0:
                    xT = xT0
                    # Startup: operands arrive at DMA rate; consume each ki
                    # chunk for two mj stripes as it lands (ki-outer, 2 open
                    # groups — same interleaving degree as the plain loop).
                    MJ_HEAD = 2
                    head = [pspool.tile([P, N], F32, tag="ps", name=f"ph{mj}")
                            for mj in range(MJ_HEAD)]
                    for ki in range(KT):
                        for mj in range(MJ_HEAD):
                            lhsT = xT[ki][:, mj, :]
                            for nj in range(NT):
                                nc.tensor.matmul(
                                    head[mj][:, nj * NB:(nj + 1) * NB], lhsT,
                                    yq[ki][:, :, nj * NB:(nj + 1) * NB],
                                    start=(ki == 0), stop=(ki == KT - 1),
                                    perf_mode=DR_SW)
                    for mj in range(MJ_HEAD):
                        ot = opool.tile([P, N], BF16, tag="o", name="oth")
                        evict(ot[:], head[mj][:], mj % 2)
                        store(o_d[0, mj * P:(mj + 1) * P, :], ot[:], mj % 2)
                    for mj in range(MJ_HEAD, MT):
                        mj_group(s, mj, xT, mj % 2)
                    continue
                else:
                    xT = []
                    for ki in range(KT):
                        xt = xpool.tile([P, MT, 2 * P], FP8, tag="xT")
                        nc.gpsimd.dma_start(out=xt[:], in_=x_d[s, ki])
                        xT.append(xt)
                for mj in range(MT):
                    mj_group(s, mj, xT, mj % 2,
                             split_evict=(s == S - 1 and mj == MT - 1))
    nc.finalize()
    return nc


def _shard_inputs(x, y):
    f8 = ml_dtypes.float8_e4m3
    in_maps = []
    for b in range(B):
        # zero points pre-applied; |values| <= 193 fit e4m3 (max 240)
        # with <= 6.25% per-element rounding error -> ~4.6e-3 rel err.
        # x shard: k-major transpose, then the DoubleRowSwInterleave weight
        # layout (see build()): per (s, ki2, mj) block of 256, position
        # 2*j + i holds column (127 - j) of k-subtile i.
        xq = (np.ascontiguousarray(x[:, b].transpose(0, 2, 1))
              - np.float32(X_ZP)).astype(f8)          # [S, K, M]
        a = xq.reshape(S, KTT, 2, P, MTT, P)          # [s, ki2, i, p, mj, j]
        a = a.transpose(0, 1, 3, 4, 5, 2)[:, :, :, :, ::-1, :]
        # y: per-ki2 DoubleRow tile layout [ki2, p, i, n] (one DMA per tile)
        yq = (y[b] - np.float32(Y_ZP)).astype(f8)    # [K, N]
        yq = yq.reshape(KTT, 2, P, N).transpose(0, 2, 1, 3)
        in_maps.append({
            "x": np.ascontiguousarray(a).reshape(S, KTT, P, MTT * 2 * P),
            "y": np.ascontiguousarray(yq),
        })
    return in_maps


def run(x, y, trace=False):
    global _CACHED_NC
    if _CACHED_NC is None:
        _CACHED_NC = build()
    nc = _CACHED_NC
    in_maps = _shard_inputs(x, y)
    res = run_bass_kernel_spmd(nc, in_maps, core_ids=list(range(B)), trace=trace)
    out = np.stack([np.asarray(res.results[b]["out"]) for b in range(B)], axis=1)
    return out.astype(np.float32), res


def kernel(x, y):
    out, _ = run(x, y, trace=False)
    return out
